# revision 3
# baseline (speedup 1.0000x reference)
"""Trainium2 Bass kernel for nn_ContextQueryAttention.

Computes, for each (batch, n_cap) pair:
    c_n = l2norm(context); q_n = l2norm(query)
    s   = (c_n @ q_n^T) / sqrt(d)          # [nw, nv]
    s_  = softmax(s, axis=v)               # masks are all-ones per the
    out = s_ @ query                       # problem spec (fill: "ones"),
                                           # so mask math is the identity.
Sharding: data-parallel over the batch dim, 4 batches per core on 8 cores.

Layout strategy per (b,c) pair (all fp32):
  - context tile [w=128, d=512] is transposed to [d, w] with the PE, using
    diag(1/||c_w||) as the matmul rhs so the transpose applies the context
    normalization for free.
  - query norm folds into the Exp activation's per-partition scale
    (s lives as s^T [v, w] with v on partitions; two pairs share the 128
    partitions).
  - softmax denominator = extra N=1 matmul vs a ones column; its reciprocal
    is applied as the per-partition scale of the mandatory PSUM->SBUF copy
    of the output.
"""

import os
import sys
from contextlib import ExitStack

os.environ.setdefault("MYCRO_LOCAL_CACHE", "1")
for _p in (
    "/root/.axon_site",
    "/root/.axon_site/_ro/trn_rl_repo",
    "/root/.axon_site/_ro/pypackages",
    "/opt/trn_rl_repo",
):
    if os.path.isdir(_p) and _p not in sys.path:
        sys.path.append(_p)

import numpy as np

import concourse.bass as bass
import concourse.tile as tile
from concourse import bacc, mybir
from concourse.bass import ts
from concourse.bass_utils import run_bass_kernel_spmd
from concourse.masks import make_identity

# Problem shapes (hardcoded; see module docstring).
BS, NCAP, NV, NW, D = 32, 20, 64, 128, 512
NCORES = 8
B_CORE = BS // NCORES          # 4 batches per core
NPAIRS = B_CORE * NCAP         # 80 (b, n_cap) pairs per core
GROUP = 4                      # pairs per processing group (1 MiB ctx DMA)
F32 = mybir.dt.float32
AF = mybir.ActivationFunctionType


def build_program(npairs=NPAIRS, group=GROUP):
    """Build (and do not compile) the single-core Bass program."""
    assert npairs % group == 0 and group % 2 == 0
    nduo = group // 2
    ngroups = npairs // group

    nc = bacc.Bacc("TRN2", target_bir_lowering=False, debug=False,
                   enable_asserts=False)
    q_d = nc.dram_tensor("q", (npairs * NV, D), F32, kind="ExternalInput").ap()
    c_d = nc.dram_tensor("c", (npairs, NW, D), F32, kind="ExternalInput").ap()
    o_d = nc.dram_tensor("o", (npairs, NW, D), F32, kind="ExternalOutput").ap()

    with tile.TileContext(nc) as tc:
        with ExitStack() as ctx:
            const = ctx.enter_context(tc.tile_pool(name="const", bufs=1))
            ident = const.tile([128, 128], F32)
            make_identity(nc, ident)
            ones = const.tile([128, 1], F32)
            nc.vector.memset(ones, 1.0)

            cin = ctx.enter_context(tc.tile_pool(name="cin", bufs=2))
            qin = ctx.enter_context(tc.tile_pool(name="qin", bufs=2))
            outp = ctx.enter_context(tc.tile_pool(name="outp", bufs=2))
            trans = ctx.enter_context(tc.tile_pool(name="trans", bufs=3))
            small = ctx.enter_context(tc.tile_pool(name="small", bufs=2))
            scr = ctx.enter_context(tc.tile_pool(name="scr", bufs=2))

            ps_qt = ctx.enter_context(tc.tile_pool(name="ps_qt", bufs=2, space="PSUM"))
            ps_ct = ctx.enter_context(tc.tile_pool(name="ps_ct", bufs=2, space="PSUM"))
            ps_s = ctx.enter_context(tc.tile_pool(name="ps_s", bufs=1, space="PSUM"))
            ps_o = ctx.enter_context(tc.tile_pool(name="ps_o", bufs=2, space="PSUM"))
            ps_den = ctx.enter_context(tc.tile_pool(name="ps_den", bufs=1, space="PSUM"))

            for g in range(ngroups):
                pg = g * group
                # ---- group loads ----
                c_sb = cin.tile([128, group, D], F32, tag="c_sb")
                nc.sync.dma_start(
                    out=c_sb, in_=c_d[pg:pg + group].rearrange("n w d -> w n d"))
                q_sb = qin.tile([128, nduo, D], F32, tag="q_sb")
                nc.sync.dma_start(
                    out=q_sb,
                    in_=q_d[pg * NV:(pg + group) * NV].rearrange(
                        "(duo p) d -> p duo d", p=128))
                out_sb = outp.tile([128, group, D], F32, tag="out_sb")

                # ---- norms ----
                # sumsq_c on ACT (Square + free-dim accumulate);
                # sumsq_q on DVE (scalar_tensor_tensor self-mult + accumulate;
                # the gpsimd variant does not lower to NEFF).
                sums_c = small.tile([128, group], F32, tag="sums_c")
                sums_q = small.tile([128, nduo], F32, tag="sums_q")
                sq_a = scr.tile([128, D], F32, tag="sq_a")
                sq_g = scr.tile([128, D], F32, tag="sq_g")
                for p_ in range(group):
                    nc.scalar.activation(out=sq_a, in_=c_sb[:, p_, :],
                                         func=AF.Square,
                                         accum_out=sums_c[:, p_:p_ + 1])
                for t in range(nduo):
                    nc.vector.scalar_tensor_tensor(
                        out=sq_g, in0=q_sb[:, t, :], scalar=1.0,
                        in1=q_sb[:, t, :],
                        op0=mybir.AluOpType.mult, op1=mybir.AluOpType.mult,
                        accum_out=sums_q[:, t:t + 1])
                norm_c = small.tile([128, group], F32, tag="norm_c")
                nc.scalar.activation(out=norm_c, in_=sums_c, func=AF.Sqrt)
                nq = small.tile([128, nduo], F32, tag="nq")
                # ||q|| * sqrt(D) == sqrt(D * sumsq_q)
                nc.scalar.activation(out=nq, in_=sums_q, func=AF.Sqrt,
                                     scale=float(D))
                inv_c = small.tile([128, group], F32, tag="inv_c")
                nc.vector.reciprocal(inv_c, norm_c)
                inv_qs = small.tile([128, nduo], F32, tag="inv_qs")
                nc.vector.reciprocal(inv_qs, nq)

                for t in range(nduo):
                    # ---- q^T via PE transpose (both pairs of the duo at once)
                    qt_ps = ps_qt.tile([128, D], F32, tag="qt_ps")
                    for j in range(4):
                        nc.tensor.transpose(qt_ps[:, ts(j, 128)],
                                            q_sb[:, t, ts(j, 128)], ident)
                    qt_sb = trans.tile([128, D], F32, tag="qt_sb")
                    nc.vector.tensor_copy(qt_sb, qt_ps)

                    # ---- normalized c^T via PE matmul with diag(inv_c) ----
                    cnt_sbs = []
                    for two in range(2):
                        p_ = t * 2 + two
                        diag = trans.tile([128, 128], F32, tag="diag")
                        nc.vector.tensor_scalar_mul(diag, ident,
                                                    inv_c[:, p_:p_ + 1])
                        cnt_ps = ps_ct.tile([128, D], F32, tag="cnt_ps")
                        for j in range(4):
                            nc.tensor.matmul(cnt_ps[:, ts(j, 128)],
                                             lhsT=c_sb[:, p_, ts(j, 128)],
                                             rhs=diag, start=True, stop=True)
                        cnt_sb = trans.tile([128, D], F32, tag="cnt_sb")
                        nc.vector.tensor_copy(cnt_sb, cnt_ps)
                        cnt_sbs.append(cnt_sb)

                    # ---- s^T = (q^T)^T @ cn^T, both pairs col-tiled ----
                    st_ps = ps_s.tile([128, 128], F32, tag="st")
                    for two in range(2):
                        for j in range(4):
                            nc.tensor.matmul(
                                st_ps[ts(two, 64), :],
                                lhsT=qt_sb[:, j * 128 + two * 64:
                                           j * 128 + two * 64 + 64],
                                rhs=cnt_sbs[two][:, ts(j, 128)],
                                start=(j == 0), stop=(j == 3),
                                tile_position=(0, two * 64))
                    # exp(s^T * inv_qs) for both pairs in one op
                    expt = trans.tile([128, 128], F32, tag="expt")
                    nc.scalar.activation(out=expt, in_=st_ps, func=AF.Exp,
                                         scale=inv_qs[:, t:t + 1])

                    # ---- out_raw = exp^T @ q ; den = exp^T @ 1 ----
                    den_ps = ps_den.tile([128, 2], F32, tag="den")
                    out_pss = []
                    for two in range(2):
                        out_ps = ps_o.tile([128, D], F32, tag="out_ps")
                        nc.tensor.matmul(out_ps, lhsT=expt[ts(two, 64), :],
                                         rhs=q_sb[ts(two, 64), t, :],
                                         start=True, stop=True,
                                         tile_position=(two * 64, 0))
                        nc.tensor.matmul(den_ps[:, two:two + 1],
                                         lhsT=expt[ts(two, 64), :],
                                         rhs=ones[ts(two, 64), :],
                                         start=True, stop=True,
                                         tile_position=(two * 64, 0))
                        out_pss.append(out_ps)
                    recip = small.tile([128, 2], F32, tag="recip")
                    nc.vector.reciprocal(recip, den_ps)
                    for two in range(2):
                        p_ = t * 2 + two
                        nc.scalar.activation(out=out_sb[:, p_, :],
                                             in_=out_pss[two], func=AF.Copy,
                                             scale=recip[:, two:two + 1])

                # ---- group store ----
                nc.sync.dma_start(
                    out=o_d[pg:pg + group].rearrange("n w d -> w n d"),
                    in_=out_sb)

    return nc


_CACHE = {}


def _compiled(npairs=NPAIRS, group=GROUP):
    key = (npairs, group)
    if key not in _CACHE:
        nc = build_program(npairs, group)
        nc.compile()
        _CACHE[key] = nc
    return _CACHE[key]


def _in_maps(query, context):
    query = np.ascontiguousarray(np.asarray(query, dtype=np.float32))
    context = np.ascontiguousarray(np.asarray(context, dtype=np.float32))
    maps = []
    for i in range(NCORES):
        qs = query[i * B_CORE:(i + 1) * B_CORE].reshape(NPAIRS * NV, D)
        cs = context[i * B_CORE:(i + 1) * B_CORE].reshape(NPAIRS, NW, D)
        maps.append({"q": qs, "c": cs})
    return maps


def _assemble(results):
    out = np.empty((BS, 1, NCAP, NW, D), dtype=np.float32)
    for i in range(NCORES):
        out[i * B_CORE:(i + 1) * B_CORE] = results[i]["o"].reshape(
            B_CORE, 1, NCAP, NW, D)
    return out


def kernel(query, query_mask, context, context_mask):
    # Masks are all-ones for this problem (spec fill: "ones") -> identity.
    nc = _compiled()
    res = run_bass_kernel_spmd(nc, _in_maps(query, context),
                               core_ids=list(range(NCORES)))
    return _assemble(res.results)


def kernel_timed(query, query_mask, context, context_mask, **trace_kwargs):
    """Like kernel() but traces core 0 and returns (out, exec_time_ns)."""
    nc = _compiled()
    res = run_bass_kernel_spmd(nc, _in_maps(query, context),
                               core_ids=list(range(NCORES)), trace=True,
                               **trace_kwargs)
    return _assemble(res.results), res.exec_time_ns
